# revision 12
# baseline (speedup 1.0000x reference)
"""Trainium2 Bass kernel for ComplexProjection:
    out[b,r,p] = |sum_s complex(x_real,x_imag)[b,r,s] * projection[r,s,p]|

Data-parallel over B across 8 NeuronCores (Bc=4096/core).

The baseline was DMA-bound (96MB/core @ ~280GB/s). This version cuts HBM
traffic by dtype engineering against the 2e-2 rel-err gate:
  - x planes quantized host-side to fp8 e3m4 (1B/elem, ~1.35% gemm err)
    or fp16 (KX=f16 safe mode, ~0.03%).
  - device computes ssum = re^2 + im^2 and writes it as fp16 (2B/elem);
    the host takes sqrt (error-free) and transposes.
  -> 16MB in + 16MB out per core (e3 mode) vs 96MB baseline.

Device dataflow per core, per r-chunk of CH=1024 particles:
  ps[:, 0:CH]    = w_r.T @ xr_chunk     (PE, fp16 w stationary)
  ps[:, CH:2CH]  = w_r.T @ xi_chunk     (same PSUM tile: 4 banks)
  epilogue patterns (mixed per-chunk to balance ACT/DVE/GPSIMD):
    combined: s1 = ACT.Square(ps[0:2CH]) -> fp16; DVE add halves -> o
    split:    ACT.Square(ps[0:CH]) -> s1a; DVE copy+mul ps[CH:2CH] -> s2
              (or DVE tensor_tensor(ps,ps) direct if KDIRECT=1);
              add on GPSIMD or DVE per pattern.

DMA: x loaded per r-group (RG=2 -> 1MB loads, sync engine ring), output
stored per r-group (2MB stores, scalar engine ring so stores don't
head-of-line-block loads on the sync HWDGE FIFO).
"""

import os

import numpy as np

B, R, S, P = 32768, 16, 128, 128
NCORES = 8
BC = B // NCORES   # 4096 particles per core
CH = 512           # matmul moving-dim chunk (PSUM: out must fit 512 fp32)
NCH = BC // CH     # 8 chunks per r
RG = 2             # r-values per x-load / out-store group
NRG = R // RG

MODE = os.environ.get("KX", "e3")        # x dtype: e3 | f16
# epilogue pattern, one char per chunk index (cycled): c=combined,
# d=split w/ DVE add, g=split w/ GPSIMD add
PAT = os.environ.get("KPAT", "cCgg")

_prog_cache = {}


def _build(nc, tile, mybir):
    f32 = mybir.dt.float32
    f16 = mybir.dt.float16
    bf16 = mybir.dt.bfloat16
    xdt = {"e3": mybir.dt.float8e3, "f16": f16}[MODE]

    xr = nc.dram_tensor("xr", [S, R, BC], xdt, kind="ExternalInput")
    xi = nc.dram_tensor("xi", [S, R, BC], xdt, kind="ExternalInput")
    w = nc.dram_tensor("w", [S, R * P], f16, kind="ExternalInput")
    o = nc.dram_tensor("o", [P, R, BC], bf16, kind="ExternalOutput")
    xr_ap, xi_ap, w_ap, o_ap = xr.ap(), xi.ap(), w.ap(), o.ap()

    with tile.TileContext(nc) as tc:
        with (
            tc.tile_pool(name="wp", bufs=1) as wp,
            tc.tile_pool(name="xp", bufs=3) as xp,
            tc.tile_pool(name="op", bufs=2) as op,
            tc.tile_pool(name="sq", bufs=4) as sqp,
            tc.tile_pool(name="ps", bufs=4, space="PSUM") as psp,
        ):
            w_sb = wp.tile([S, R * P], f16)
            nc.sync.dma_start(w_sb[:], w_ap[:])

            # HAM warmup: ~6us of junk matmuls (dep: w only) so the PE
            # clock-gate reaches 8/8 before the first real chunk. Uses a
            # buffer of the main PSUM ring; its result is never read.
            wu = psp.tile([P, 2 * CH], f32, tag="ps")
            for k in range(16):
                nc.tensor.matmul(wu[:, 0:CH], w_sb[:, 0:P], w_sb[:, 0:CH],
                                 start=True, stop=True)

            for rg in range(NRG):
                rsl = slice(rg * RG, (rg + 1) * RG)
                xr_sb = xp.tile([S, RG, BC], xdt, tag="xr")
                xi_sb = xp.tile([S, RG, BC], xdt, tag="xi")
                if rg == 0:
                    # finer first loads so the first matmuls start early
                    for rr in range(RG):
                        nc.sync.dma_start(xr_sb[:, rr, :],
                                          xr_ap[:, rg * RG + rr, :])
                        nc.sync.dma_start(xi_sb[:, rr, :],
                                          xi_ap[:, rg * RG + rr, :])
                else:
                    nc.sync.dma_start(xr_sb[:], xr_ap[:, rsl, :])
                    nc.sync.dma_start(xi_sb[:], xi_ap[:, rsl, :])
                o_sb = op.tile([P, RG, BC], bf16, tag="o")
                for rr in range(RG):
                    r = rg * RG + rr
                    w_r = w_sb[:, r * P:(r + 1) * P]
                    for cc in range(NCH):
                        sl = slice(cc * CH, (cc + 1) * CH)
                        ps = psp.tile([P, 2 * CH], f32, tag="ps")
                        nc.tensor.matmul(ps[:, 0:CH], w_r, xr_sb[:, rr, sl],
                                         start=True, stop=True)
                        nc.tensor.matmul(ps[:, CH:2 * CH], w_r,
                                         xi_sb[:, rr, sl],
                                         start=True, stop=True)
                        kind = PAT[(rr * NCH + cc) % len(PAT)]
                        osl = o_sb[:, rr, sl]
                        if kind in "cC":
                            s1 = sqp.tile([P, 2 * CH], bf16, tag="s1")
                            nc.scalar.square(s1[:], ps[:])
                            if kind == "C":
                                nc.gpsimd.tensor_add(osl, s1[:, 0:CH],
                                                     s1[:, CH:2 * CH])
                            else:
                                nc.vector.tensor_add(osl, s1[:, 0:CH],
                                                     s1[:, CH:2 * CH])
                        else:
                            s1a = sqp.tile([P, CH], bf16, tag="s1a")
                            nc.scalar.square(s1a[:], ps[:, 0:CH])
                            s2 = sqp.tile([P, CH], bf16, tag="s2")
                            c = sqp.tile([P, CH], bf16, tag="cp")
                            nc.vector.tensor_copy(c[:], ps[:, CH:2 * CH])
                            nc.vector.tensor_mul(s2[:], c[:], c[:])
                            if kind == "g":
                                nc.gpsimd.tensor_add(osl, s1a[:], s2[:])
                            else:
                                nc.vector.tensor_add(osl, s1a[:], s2[:])
                nc.scalar.dma_start(o_ap[:, rsl, :], o_sb[:])


def _build_program():
    key = (MODE, PAT)
    if key in _prog_cache:
        return _prog_cache[key]

    import concourse.tile as tile
    from concourse import bacc, mybir

    nc = bacc.Bacc("TRN2", target_bir_lowering=False, debug=False,
                   num_devices=NCORES)
    _build(nc, tile, mybir)
    nc.compile()
    _prog_cache[key] = nc
    return nc


LAST_RESULT = None


def kernel(x_real, x_imag, projection):
    global LAST_RESULT
    import ml_dtypes
    from concourse.bass_utils import run_bass_kernel_spmd

    nc = _build_program()
    xdt = {"e3": ml_dtypes.float8_e3m4, "f16": np.float16}[MODE]

    x_real = np.ascontiguousarray(x_real, dtype=np.float32)
    x_imag = np.ascontiguousarray(x_imag, dtype=np.float32)
    w = np.ascontiguousarray(projection, dtype=np.float32)
    # device expects w as [s, r*p] fp16
    w16 = np.ascontiguousarray(
        w.transpose(1, 0, 2).reshape(S, R * P)).astype(np.float16)

    in_maps = []
    for c in range(NCORES):
        sl = slice(c * BC, (c + 1) * BC)
        # (BC, R, S) -> (S, R, BC)
        xr_t = x_real[sl].transpose(2, 1, 0).astype(xdt)
        xi_t = x_imag[sl].transpose(2, 1, 0).astype(xdt)
        in_maps.append({"xr": np.ascontiguousarray(xr_t),
                        "xi": np.ascontiguousarray(xi_t),
                        "w": w16})

    res = run_bass_kernel_spmd(nc, in_maps, core_ids=list(range(NCORES)))
    LAST_RESULT = res
    out = np.empty((B, R, P), dtype=np.float32)
    for c in range(NCORES):
        ssum = np.asarray(res.results[c]["o"]).astype(np.float32)  # (P, R, BC)
        out[c * BC:(c + 1) * BC] = np.sqrt(ssum).transpose(2, 1, 0)
    return out


# revision 13
# speedup vs baseline: 1.1484x; 1.1484x over previous
"""Trainium2 Bass kernel for ComplexProjection:
    out[b,r,p] = |sum_s complex(x_real,x_imag)[b,r,s] * projection[r,s,p]|

Data-parallel over B across 8 NeuronCores (Bc=4096/core).

The baseline was DMA-bound (96MB/core @ ~280GB/s). This version cuts HBM
traffic by dtype engineering against the 2e-2 rel-err gate:
  - x planes quantized host-side to fp8 e3m4 (1B/elem, ~1.35% gemm err)
    or fp16 (KX=f16 safe mode, ~0.03%).
  - device computes ssum = re^2 + im^2 and writes it as fp16 (2B/elem);
    the host takes sqrt (error-free) and transposes.
  -> 16MB in + 16MB out per core (e3 mode) vs 96MB baseline.

Device dataflow per core, per r-chunk of CH=1024 particles:
  ps[:, 0:CH]    = w_r.T @ xr_chunk     (PE, fp16 w stationary)
  ps[:, CH:2CH]  = w_r.T @ xi_chunk     (same PSUM tile: 4 banks)
  epilogue patterns (mixed per-chunk to balance ACT/DVE/GPSIMD):
    combined: s1 = ACT.Square(ps[0:2CH]) -> fp16; DVE add halves -> o
    split:    ACT.Square(ps[0:CH]) -> s1a; DVE copy+mul ps[CH:2CH] -> s2
              (or DVE tensor_tensor(ps,ps) direct if KDIRECT=1);
              add on GPSIMD or DVE per pattern.

DMA: x loaded per r-group (RG=2 -> 1MB loads, sync engine ring), output
stored per r-group (2MB stores, scalar engine ring so stores don't
head-of-line-block loads on the sync HWDGE FIFO).
"""

import os

import numpy as np

B, R, S, P = 32768, 16, 128, 128
NCORES = 8
BC = B // NCORES   # 4096 particles per core
CH = 512           # matmul moving-dim chunk (PSUM: out must fit 512 fp32)
NCH = BC // CH     # 8 chunks per r
RG = 2             # r-values per x-load / out-store group
NRG = R // RG

MODE = os.environ.get("KX", "e3")        # x dtype: e3 | f16
# epilogue pattern, one char per chunk index (cycled): c=combined,
# d=split w/ DVE add, g=split w/ GPSIMD add
PAT = os.environ.get("KPAT", "cg")

_prog_cache = {}


def _build(nc, tile, mybir):
    f32 = mybir.dt.float32
    f16 = mybir.dt.float16
    bf16 = mybir.dt.bfloat16
    xdt = {"e3": mybir.dt.float8e3, "f16": f16}[MODE]

    xr = nc.dram_tensor("xr", [S, R, BC], xdt, kind="ExternalInput")
    xi = nc.dram_tensor("xi", [S, R, BC], xdt, kind="ExternalInput")
    w = nc.dram_tensor("w", [S, R * P], f16, kind="ExternalInput")
    o = nc.dram_tensor("o", [P, R, BC], bf16, kind="ExternalOutput")
    xr_ap, xi_ap, w_ap, o_ap = xr.ap(), xi.ap(), w.ap(), o.ap()

    with tile.TileContext(nc) as tc:
        with (
            tc.tile_pool(name="wp", bufs=1) as wp,
            tc.tile_pool(name="xp", bufs=3) as xp,
            tc.tile_pool(name="op", bufs=2) as op,
            tc.tile_pool(name="sq", bufs=4) as sqp,
            tc.tile_pool(name="ps", bufs=4, space="PSUM") as psp,
        ):
            w_sb = wp.tile([S, R * P], f16)
            nc.sync.dma_start(w_sb[:], w_ap[:])

            # HAM warmup: ~6us of junk matmuls (dep: w only) so the PE
            # clock-gate reaches 8/8 before the first real chunk. Uses a
            # buffer of the main PSUM ring; its result is never read.
            wu = psp.tile([P, 2 * CH], f32, tag="ps")
            for k in range(16):
                nc.tensor.matmul(wu[:, 0:CH], w_sb[:, 0:P], w_sb[:, 0:CH],
                                 start=True, stop=True)

            for rg in range(NRG):
                rsl = slice(rg * RG, (rg + 1) * RG)
                xr_sb = xp.tile([S, RG, BC], xdt, tag="xr")
                xi_sb = xp.tile([S, RG, BC], xdt, tag="xi")
                if rg == 0:
                    # finer first loads so the first matmuls start early
                    for rr in range(RG):
                        nc.sync.dma_start(xr_sb[:, rr, :],
                                          xr_ap[:, rg * RG + rr, :])
                        nc.sync.dma_start(xi_sb[:, rr, :],
                                          xi_ap[:, rg * RG + rr, :])
                else:
                    nc.sync.dma_start(xr_sb[:], xr_ap[:, rsl, :])
                    nc.sync.dma_start(xi_sb[:], xi_ap[:, rsl, :])
                o_sb = op.tile([P, RG, BC], bf16, tag="o")
                for rr in range(RG):
                    r = rg * RG + rr
                    w_r = w_sb[:, r * P:(r + 1) * P]
                    for cc in range(NCH):
                        sl = slice(cc * CH, (cc + 1) * CH)
                        ps = psp.tile([P, 2 * CH], f32, tag="ps")
                        nc.tensor.matmul(ps[:, 0:CH], w_r, xr_sb[:, rr, sl],
                                         start=True, stop=True)
                        nc.tensor.matmul(ps[:, CH:2 * CH], w_r,
                                         xi_sb[:, rr, sl],
                                         start=True, stop=True)
                        kind = PAT[(rr * NCH + cc) % len(PAT)]
                        osl = o_sb[:, rr, sl]
                        if kind in "cC":
                            s1 = sqp.tile([P, 2 * CH], bf16, tag="s1")
                            nc.scalar.square(s1[:], ps[:])
                            if kind == "C":
                                nc.gpsimd.tensor_add(osl, s1[:, 0:CH],
                                                     s1[:, CH:2 * CH])
                            else:
                                nc.vector.tensor_add(osl, s1[:, 0:CH],
                                                     s1[:, CH:2 * CH])
                        else:
                            s1a = sqp.tile([P, CH], bf16, tag="s1a")
                            nc.scalar.square(s1a[:], ps[:, 0:CH])
                            s2 = sqp.tile([P, CH], bf16, tag="s2")
                            c = sqp.tile([P, CH], bf16, tag="cp")
                            nc.vector.tensor_copy(c[:], ps[:, CH:2 * CH])
                            nc.vector.tensor_mul(s2[:], c[:], c[:])
                            if kind == "g":
                                nc.gpsimd.tensor_add(osl, s1a[:], s2[:])
                            else:
                                nc.vector.tensor_add(osl, s1a[:], s2[:])
                nc.scalar.dma_start(o_ap[:, rsl, :], o_sb[:])


def _build_program():
    key = (MODE, PAT)
    if key in _prog_cache:
        return _prog_cache[key]

    import concourse.tile as tile
    from concourse import bacc, mybir

    nc = bacc.Bacc("TRN2", target_bir_lowering=False, debug=False,
                   num_devices=NCORES)
    _build(nc, tile, mybir)
    nc.compile()
    _prog_cache[key] = nc
    return nc


LAST_RESULT = None


def kernel(x_real, x_imag, projection):
    global LAST_RESULT
    import ml_dtypes
    from concourse.bass_utils import run_bass_kernel_spmd

    nc = _build_program()
    xdt = {"e3": ml_dtypes.float8_e3m4, "f16": np.float16}[MODE]

    x_real = np.ascontiguousarray(x_real, dtype=np.float32)
    x_imag = np.ascontiguousarray(x_imag, dtype=np.float32)
    w = np.ascontiguousarray(projection, dtype=np.float32)
    # device expects w as [s, r*p] fp16
    w16 = np.ascontiguousarray(
        w.transpose(1, 0, 2).reshape(S, R * P)).astype(np.float16)

    in_maps = []
    for c in range(NCORES):
        sl = slice(c * BC, (c + 1) * BC)
        # (BC, R, S) -> (S, R, BC)
        xr_t = x_real[sl].transpose(2, 1, 0).astype(xdt)
        xi_t = x_imag[sl].transpose(2, 1, 0).astype(xdt)
        in_maps.append({"xr": np.ascontiguousarray(xr_t),
                        "xi": np.ascontiguousarray(xi_t),
                        "w": w16})

    res = run_bass_kernel_spmd(nc, in_maps, core_ids=list(range(NCORES)))
    LAST_RESULT = res
    out = np.empty((B, R, P), dtype=np.float32)
    for c in range(NCORES):
        ssum = np.asarray(res.results[c]["o"]).astype(np.float32)  # (P, R, BC)
        out[c * BC:(c + 1) * BC] = np.sqrt(ssum).transpose(2, 1, 0)
    return out
